# revision 1
# baseline (speedup 1.0000x reference)
"""Distributed SigLIP loss kernel for 8 trn2 NeuronCores.

loss*N = sum_ij softplus(L_ij) - sum_i L_ii,   L = exp(t')*(x_hat @ y_hat.T) + b

Sharding: img rows split 8 ways (2048 rows/core); every core holds full txt.
Per-core out-tiles are [128 txt-rows (partitions), 2048 img-cols (free)] so the
txt-row inv-norm rides the ACT per-partition scale and the img inv-norm (and
exp(t')) is pre-multiplied into the moving matmul operand.

softplus is composed as exp (ACT) -> pair-merge (DVE, w = ea*eb + ea + eb) ->
ln(w+1) (ACT, same exp/ln table set, accum_out per-partition reduction).
The host only reshapes/transposes/replicates inputs; all math runs on device.
"""

import sys
from contextlib import ExitStack

import numpy as np

try:
    import concourse.bass as bass  # noqa: F401
except ImportError:  # pragma: no cover
    sys.path.append("/opt/trn_rl_repo")
    import concourse.bass as bass  # noqa: F401

import concourse.mybir as mybir
import concourse.tile as tile
from concourse import bacc
from concourse.bass_utils import run_bass_kernel_spmd

# Keep Exp/Ln resolvable only via the combined natural_log_exp set, and Sqrt
# via a single set, so Bacc's table-load pass doesn't ping-pong table loads
# between exp-only and ln-only sets inside the main loop. Dict order (the
# act_func_set_id space) is preserved; only membership used for selection is
# narrowed, and the runtime tables genuinely contain the functions.
import functools as _functools

import concourse.hw_specs as _hw_specs


_ORIG_ACT_TABLES = _hw_specs.get_activation_tables.__wrapped__


@_functools.cache
def _patched_act_tables(module_arch):
    orig = _ORIG_ACT_TABLES(module_arch)
    _AF = mybir.ActivationFunctionType
    patched = {}
    for name, funcs in orig.items():
        funcs = set(funcs)
        if name != "natural_log_exp_and_others":
            funcs.discard(_AF.Exp)
            funcs.discard(_AF.Ln)
        if name != "sqrt_and_others":
            funcs.discard(_AF.Sqrt)
        patched[name] = funcs
    return patched


_hw_specs.get_activation_tables = _patched_act_tables
if getattr(bacc, "get_activation_tables", None) is not None:
    bacc.get_activation_tables = _patched_act_tables

N = 16384
D = 256
CORES = 8
SH = N // CORES          # 2048 img rows per core
NT = N // 128            # 128 txt-row tiles (out-tiles) per core
MT = SH // 128           # 16 row-tiles in the shard
LN_BATCH = 8             # out-tiles per ln instruction -> [128, 8192]
F32 = mybir.dt.float32
F16 = mybir.dt.float16
ADD = mybir.AluOpType.add
MULT = mybir.AluOpType.mult
SUB = mybir.AluOpType.subtract
AF = mybir.ActivationFunctionType

_CACHED_NC = None


def _build_nc():
    nc = bacc.Bacc(
        "TRN2",
        target_bir_lowering=False,
        debug=False,
        enable_asserts=False,
        num_devices=CORES,
    )
    txtT = nc.dram_tensor("txtT", [D, N], F16, kind="ExternalInput").ap()
    txtRF = nc.dram_tensor("txtRF", [N, D], F16, kind="ExternalInput").ap()
    txtRsh = nc.dram_tensor("txtRsh", [SH, D], F16, kind="ExternalInput").ap()
    imgT = nc.dram_tensor("imgT", [D, SH], F16, kind="ExternalInput").ap()
    imgR = nc.dram_tensor("imgR", [SH, D], F16, kind="ExternalInput").ap()
    tp = nc.dram_tensor("tp", [128, 1], F32, kind="ExternalInput").ap()
    bs = nc.dram_tensor("bs", [128, 1], F32, kind="ExternalInput").ap()
    out = nc.dram_tensor("out", [1, 1], F32, kind="ExternalOutput").ap()

    with tile.TileContext(nc) as tc, ExitStack() as ctx:
        big = ctx.enter_context(tc.tile_pool(name="big", bufs=1))
        rows = ctx.enter_context(tc.tile_pool(name="rows", bufs=32))
        scrp = ctx.enter_context(tc.tile_pool(name="scrp", bufs=4))
        ep = ctx.enter_context(tc.tile_pool(name="ep", bufs=3))
        uvp = ctx.enter_context(tc.tile_pool(name="uvp", bufs=3))
        lnp = ctx.enter_context(tc.tile_pool(name="lnp", bufs=2))
        small = ctx.enter_context(tc.tile_pool(name="small", bufs=1))
        psum = ctx.enter_context(tc.tile_pool(name="psum", bufs=2, space="PSUM"))

        # ---- resident loads --------------------------------------------
        # scalar (ACT) queue: small per-core inputs needed earliest
        tp_sb = small.tile([128, 1], F32, tag="tp")
        nc.scalar.dma_start(tp_sb[:], tp[:])
        bs_sb = small.tile([128, 1], F32, tag="bs")
        nc.scalar.dma_start(bs_sb[:], bs[:])
        imgT_sb = []
        for k in range(2):
            t = big.tile([128, SH], F16, tag=f"imgT{k}")
            nc.scalar.dma_start(t[:], imgT[128 * k : 128 * (k + 1), :])
            imgT_sb.append(t)
        # sync (SP) queue: imgR (needed first for scales), big txtT tiles,
        # then the norm-row stream
        imgR_sb = []
        for j in range(MT):
            r = big.tile([128, D], F16, tag=f"imgR{j}")
            nc.sync.dma_start(r[:], imgR[128 * j : 128 * (j + 1), :])
            imgR_sb.append(r)
        # txtT k0 on sync, k1 on scalar: the two 4MB loads ride different
        # HWDGE queues so tile-0's matmuls (which need both k-chunks) start
        # ~15us earlier
        txtT_sb = []
        for k, eng in ((0, nc.sync), (1, nc.scalar)):
            t = big.tile([128, N], F16, tag=f"txtT{k}")
            eng.dma_start(t[:], txtT[128 * k : 128 * (k + 1), :])
            txtT_sb.append(t)

        e_ap = small.tile([128, 1], F32, tag="eap")
        nc.scalar.activation(e_ap[:], tp_sb[:], AF.Exp)

        ones_col = small.tile([128, 1], F32, tag="onesc")
        nc.vector.memset(ones_col[:], 1.0)
        ones_row16 = small.tile([1, 128], F16, tag="onesr16")
        nc.vector.memset(ones_row16[:], 1.0)
        ident = small.tile([128, 128], F32, tag="ident")
        from concourse.masks import make_identity

        make_identity(nc, ident[:])

        # ---- img scales: norms from imgR rows (shared with the diagonal),
        # 1/sqrt(a) = exp(-0.5*ln(a)) keeps everything in the exp/ln table
        # set; the exp(t') factor folds into the Exp bias.
        nsqx = small.tile([128, MT], F32, tag="nsqx")
        for j in range(MT):
            s1 = scrp.tile([128, D], F32, tag="dscr")
            nc.vector.scalar_tensor_tensor(
                s1[:], imgR_sb[j][:], 1.0, imgR_sb[j][:], op0=MULT, op1=MULT,
                accum_out=nsqx[:, j : j + 1],
            )
        lx = small.tile([128, MT], F32, tag="lx")
        nc.scalar.activation(lx[:], nsqx[:], AF.Ln)
        sxm = small.tile([128, MT], F32, tag="sxm")
        nc.scalar.activation(
            sxm[:], lx[:], AF.Exp, bias=tp_sb[:], scale=-0.5
        )
        ix = small.tile([128, MT], F32, tag="ix")
        nc.scalar.activation(ix[:], lx[:], AF.Exp, scale=-0.5)
        # transpose to free layout and broadcast across partitions via PE
        sxm_ps = psum.tile([16, 128], F32, tag="mm")
        nc.tensor.transpose(sxm_ps[:], sxm[:], ident[:])
        s16 = small.tile([16, 128], F16, tag="s16")
        nc.vector.tensor_copy(s16[:], sxm_ps[:])
        s1row = small.tile([1, SH], F16, tag="s1row")
        nc.gpsimd.dma_start(s1row[:], s16[:])
        sb_ps = psum.tile([128, SH], F32, tag="mm")
        for g in range(SH // 512):
            nc.tensor.matmul(
                sb_ps[:, 512 * g : 512 * (g + 1)],
                lhsT=ones_row16[:],
                rhs=s1row[:, 512 * g : 512 * (g + 1)],
                start=True,
                stop=True,
            )
        imgTs = []
        for k in range(2):
            t = big.tile([128, SH], F16, tag=f"imgTs{k}")
            nc.vector.tensor_tensor(t[:], imgT_sb[k][:], sb_ps[:], op=MULT)
            imgTs.append(t)

        # ---- main loop, with txt-norm chunks interleaved ----------------
        # txt norms stream on the sync queue behind the txtT loads; inv-norm
        # chunks are computed right before the 16 exps that consume them so
        # the ACT FIFO never blocks on far-future norms.
        NCHUNK = 16
        nsq = small.tile([128, NT], F32, tag="nsq")
        rinv_n = small.tile([128, NT], F32, tag="rinvn")
        invn = small.tile([128, NT], F32, tag="invn")
        acc = small.tile([128, NT // LN_BATCH], F32, tag="acc")
        half = SH // 2
        quart = SH // 4
        eighth = SH // 8
        lnt = None
        pending_ln = None

        def _norm_stt(u):
            rt = rows.tile([128, D], F16, tag="trow")
            nc.sync.dma_start(rt[:], txtRF[128 * u : 128 * (u + 1), :])
            scr = scrp.tile([128, D], F32, tag="ttrs")
            nc.vector.scalar_tensor_tensor(
                scr[:], rt[:], 1.0, rt[:], op0=MULT, op1=MULT,
                accum_out=nsq[:, u : u + 1],
            )

        def _finish_norm_chunk(c):
            cs = slice(NCHUNK * c, NCHUNK * (c + 1))
            nc.scalar.activation(rinv_n[:, cs], nsq[:, cs], AF.Ln)
            nc.scalar.activation(invn[:, cs], rinv_n[:, cs], AF.Exp, scale=-0.5)

        for u in range(NCHUNK):
            _norm_stt(u)
        _finish_norm_chunk(0)
        for t in range(NT):
            # stream the next chunk's norm reductions two per tile across
            # tiles 4..11 of the current chunk, finish at tile 12 so the
            # chunk's Ln/Exp never sits at the ACT FIFO head
            h = t % NCHUNK - 4
            c_next = t // NCHUNK + 1
            if 0 <= h < 8 and c_next < NT // NCHUNK:
                u0 = NCHUNK * c_next + 2 * h
                _norm_stt(u0)
                _norm_stt(u0 + 1)
            if t % NCHUNK == 12 and c_next < NT // NCHUNK:
                _finish_norm_chunk(c_next)
            ps = psum.tile([128, SH], F32, tag="mm")
            for k in range(2):
                for j in range(SH // 512):
                    sl = slice(512 * j, 512 * (j + 1))
                    nc.tensor.matmul(
                        ps[:, sl],
                        lhsT=txtT_sb[k][:, 128 * t : 128 * (t + 1)],
                        rhs=imgTs[k][:, sl],
                        start=(k == 0),
                        stop=(k == 1),
                    )
            et = ep.tile([128, SH], F16, tag="et")
            nc.scalar.activation(
                et[:], ps[:], AF.Exp, bias=bs_sb[:], scale=invn[:, t : t + 1]
            )
            if pending_ln is not None and t % LN_BATCH == 2:
                pending_ln()
                pending_ln = None
            if t % LN_BATCH == 0:
                lnt = lnp.tile([128, eighth * LN_BATCH], F16, tag="lnt")
            # p = (1+e^l)/4 (two-op tensor_scalar, 4x), then three
            # pair-product levels (tensor_tensor, 2x): ln of the 8-way
            # product sums 8 softplus terms minus the constant 16*ln2
            # (corrected exactly in the final reduction); the /4 keeps all
            # products within fp16 range.
            p = uvp.tile([128, SH], F16, tag="p")
            nc.vector.tensor_scalar(p[:], et[:], 1.0, 0.25, op0=ADD, op1=MULT)
            m1 = uvp.tile([128, half], F16, tag="m1")
            nc.vector.tensor_tensor(m1[:], p[:, 0:half], p[:, half:SH], op=MULT)
            m2 = uvp.tile([128, quart], F16, tag="m2")
            nc.vector.tensor_tensor(m2[:], m1[:, 0:quart], m1[:, quart:half], op=MULT)
            msl = lnt[:, eighth * (t % LN_BATCH) : eighth * (t % LN_BATCH + 1)]
            nc.vector.tensor_tensor(msl, m2[:, 0:eighth], m2[:, eighth:quart], op=MULT)
            if t % LN_BATCH == LN_BATCH - 1:
                b_idx = t // LN_BATCH
                cur_lnt = lnt

                def _emit_ln(cur_lnt=cur_lnt, b_idx=b_idx):
                    nc.scalar.activation(
                        cur_lnt[:],
                        cur_lnt[:],
                        AF.Ln,
                        accum_out=acc[:, b_idx : b_idx + 1],
                    )

                pending_ln = _emit_ln
        if pending_ln is not None:
            pending_ln()
            pending_ln = None

        # ---- shard diagonal terms (tail; overlaps main-loop drain) -----
        nsqy = small.tile([128, MT], F32, tag="nsqy")
        dots = small.tile([128, MT], F32, tag="dots")
        for j in range(MT):
            tr = scrp.tile([128, D], F16, tag="srow_t")
            nc.sync.dma_start(tr[:], txtRsh[128 * j : 128 * (j + 1), :])
            s2 = scrp.tile([128, D], F32, tag="dscr")
            nc.vector.scalar_tensor_tensor(
                s2[:], tr[:], 1.0, tr[:], op0=MULT, op1=MULT,
                accum_out=nsqy[:, j : j + 1],
            )
            s3 = scrp.tile([128, D], F32, tag="dscr")
            nc.vector.scalar_tensor_tensor(
                s3[:], imgR_sb[j][:], 1.0, tr[:], op0=MULT, op1=MULT,
                accum_out=dots[:, j : j + 1],
            )
        ry = small.tile([128, MT], F32, tag="ry")
        nc.scalar.activation(ry[:], nsqy[:], AF.Ln)
        iy = small.tile([128, MT], F32, tag="iy")
        nc.scalar.activation(iy[:], ry[:], AF.Exp, scale=-0.5)
        sim = small.tile([128, MT], F32, tag="sim")
        nc.vector.tensor_tensor(sim[:], dots[:], ix[:], op=MULT)
        sim2 = small.tile([128, MT], F32, tag="sim2")
        nc.vector.tensor_tensor(sim2[:], sim[:], iy[:], op=MULT)
        dsum = small.tile([128, 1], F32, tag="dsum")
        nc.vector.reduce_sum(dsum[:], sim2[:], axis=mybir.AxisListType.X)

        # ---- final reduction -------------------------------------------
        A = small.tile([128, 1], F32, tag="A")
        nc.vector.reduce_sum(A[:], acc[:], axis=mybir.AxisListType.X)
        t1 = small.tile([128, 1], F32, tag="t1")
        nc.vector.tensor_tensor(t1[:], dsum[:], e_ap[:], op=MULT)
        t2 = small.tile([128, 1], F32, tag="t2")
        nc.vector.tensor_scalar(t2[:], bs_sb[:], float(MT), None, op0=MULT)
        t3 = small.tile([128, 1], F32, tag="t3")
        nc.vector.tensor_tensor(t3[:], t1[:], t2[:], op=ADD)
        corr = float((SH * N / 128 / 8) * 16.0 * np.log(2.0))
        A2 = small.tile([128, 1], F32, tag="A2")
        nc.vector.tensor_scalar(A2[:], A[:], corr, None, op0=ADD)
        C = small.tile([128, 1], F32, tag="C")
        nc.vector.tensor_tensor(C[:], A2[:], t3[:], op=SUB)
        fin_ps = psum.tile([1, 1], F32, tag="mm")
        nc.tensor.matmul(
            fin_ps[:], lhsT=ones_col[:], rhs=C[:], start=True, stop=True
        )
        fin = small.tile([1, 1], F32, tag="fin")
        nc.vector.tensor_copy(fin[:], fin_ps[:])
        nc.sync.dma_start(out[:], fin[:])

    nc.compile()
    return nc


def _get_nc():
    global _CACHED_NC
    if _CACHED_NC is None:
        _CACHED_NC = _build_nc()
    return _CACHED_NC


def _make_in_maps(img, txt, t_prime, bias):
    img = np.asarray(img, dtype=np.float32)
    txt = np.asarray(txt, dtype=np.float32)
    tpv = float(np.asarray(t_prime, dtype=np.float32))
    bsv = float(np.asarray(bias, dtype=np.float32))

    txt16 = txt.astype(np.float16)
    txtT = np.ascontiguousarray(txt16.T)            # [D, N]
    img16 = img.astype(np.float16)
    imgT = np.ascontiguousarray(img16.T)            # [D, N]

    tp_arr = np.full((128, 1), tpv, dtype=np.float32)
    bs_arr = np.full((128, 1), bsv, dtype=np.float32)

    in_maps = []
    for c in range(CORES):
        sl = slice(SH * c, SH * (c + 1))
        in_maps.append(
            {
                "txtT": txtT,
                "txtRF": txt16,
                "txtRsh": np.ascontiguousarray(txt16[sl]),
                "imgT": np.ascontiguousarray(imgT[:, sl]),
                "imgR": np.ascontiguousarray(img16[sl]),
                "tp": tp_arr,
                "bs": bs_arr,
            }
        )
    return in_maps


def _run(img, txt, t_prime, bias, trace=False):
    nc = _get_nc()
    in_maps = _make_in_maps(img, txt, t_prime, bias)
    res = run_bass_kernel_spmd(
        nc, in_maps, core_ids=list(range(CORES)), trace=trace
    )
    partials = [float(r["out"][0, 0]) for r in res.results]
    loss = np.float32(sum(partials) / N)
    return loss, res


def kernel(img, txt, t_prime, bias):
    loss, _ = _run(img, txt, t_prime, bias, trace=False)
    return np.asarray(loss, dtype=np.float32)



# revision 12
# speedup vs baseline: 1.0307x; 1.0307x over previous
"""Distributed SigLIP loss kernel for 8 trn2 NeuronCores.

loss*N = sum_ij softplus(L_ij) - sum_i L_ii,  L = s*c + b,  s = exp(t'),
c_ij = cos(x_i, y_j) in [-1, 1] always (Cauchy-Schwarz).

Fast path (moment method): softplus(s*c+b) is approximated by a degree-2
Chebyshev polynomial P(c) = a0 + a1*c + a2*c^2 fitted at runtime on the
host over the FULL possible range c in [-R, R] (R slightly above 1 for
rounding slop). The fit error eps is measured on a dense grid, giving a
certified bound |sum_ij P(c_ij) - sum_ij softplus(L_ij)| <= eps*N^2 that
holds for ANY inputs (no distributional assumption). When that bound is
comfortably inside the rel-err budget, the pairwise sum collapses into
moments that never materialize the N x N logits:

    sum_ij c_ij   = (sum_i xh_i) . (sum_j yh_j)
    sum_ij c_ij^2 = < Xh^T Xh , Yh^T Yh >   (two D x D Grams)

Each core gets a 2048-row shard of img AND txt, normalizes rows on
device, and computes partial Grams on the PE (a ones-column appended to
the scaled tiles yields the row-sum vectors for free), plus the exact
diagonal dots x̂_i.ŷ_i. The host combines the 8 partials in float64.

If the certified bound is too large for the runtime (t', b) — e.g. a
very large temperature making softplus too curved over [-s+b, s+b] —
kernel() falls back to the exact exp/product-tree/ln kernel below.
"""

import sys
from contextlib import ExitStack

import numpy as np

try:
    import concourse.bass as bass  # noqa: F401
except ImportError:  # pragma: no cover
    sys.path.append("/opt/trn_rl_repo")
    import concourse.bass as bass  # noqa: F401

import concourse.mybir as mybir
import concourse.tile as tile
from concourse import bacc
from concourse.bass_utils import run_bass_kernel_spmd

# Keep Exp/Ln resolvable only via the combined natural_log_exp set, and Sqrt
# via a single set, so Bacc's table-load pass doesn't ping-pong table loads
# between exp-only and ln-only sets inside the main loop. Dict order (the
# act_func_set_id space) is preserved; only membership used for selection is
# narrowed, and the runtime tables genuinely contain the functions.
import functools as _functools

import concourse.hw_specs as _hw_specs


_ORIG_ACT_TABLES = _hw_specs.get_activation_tables.__wrapped__


@_functools.cache
def _patched_act_tables(module_arch):
    orig = _ORIG_ACT_TABLES(module_arch)
    _AF = mybir.ActivationFunctionType
    patched = {}
    for name, funcs in orig.items():
        funcs = set(funcs)
        if name != "natural_log_exp_and_others":
            funcs.discard(_AF.Exp)
            funcs.discard(_AF.Ln)
        if name != "sqrt_and_others":
            funcs.discard(_AF.Sqrt)
        patched[name] = funcs
    return patched


_hw_specs.get_activation_tables = _patched_act_tables
if getattr(bacc, "get_activation_tables", None) is not None:
    bacc.get_activation_tables = _patched_act_tables

N = 16384
D = 256
CORES = 8
SH = N // CORES          # 2048 rows of each of img/txt per core
JT = SH // 128           # 16 row-tiles per shard
NT = N // 128            # tiles for the exact path
MT = SH // 128
LN_BATCH = 8
F32 = mybir.dt.float32
F16 = mybir.dt.float16
ADD = mybir.AluOpType.add
MULT = mybir.AluOpType.mult
SUB = mybir.AluOpType.subtract
AF = mybir.ActivationFunctionType

_CACHED_MOMENT_NC = None
_CACHED_EXACT_NC = None

# certified-bound gate: use the moment path only when the Chebyshev fit
# error alone cannot push the relative error past this (budget is 2e-2)
_REL_GATE = 2e-3
_CHEB_R = 1.002  # fit range: |c| <= 1 plus fp16/PE rounding slop


# ---------------------------------------------------------------------------
# fast path: degree-2 moment kernel
# ---------------------------------------------------------------------------
def _build_moment_nc():
    nc = bacc.Bacc(
        "TRN2",
        target_bir_lowering=False,
        debug=False,
        enable_asserts=False,
        num_devices=CORES,
    )
    # partition-major layout: host pre-transposes so each DMA is a simple
    # 2D contiguous slice (cheap descriptors, no ACT-sequencer clogging)
    xin = nc.dram_tensor("xin", [128, JT * D], F16, kind="ExternalInput").ap()
    yin = nc.dram_tensor("yin", [128, JT * D], F16, kind="ExternalInput").ap()
    # [256 gram rows, 256 gram cols + 1 row-sum col]
    gxo = nc.dram_tensor("gxo", [D, D + 1], F32, kind="ExternalOutput").ap()
    gyo = nc.dram_tensor("gyo", [D, D + 1], F32, kind="ExternalOutput").ap()
    # per-group diagonal partial sums (include the ones-columns: host
    # subtracts the known CH constant per entry); host reduces
    dto = nc.dram_tensor("dto", [128, JT // 4], F32, kind="ExternalOutput").ap()

    W = D + 1   # scaled tile width: 256 data cols + ones col
    CH = 4      # tiles per DMA / norm-chunk / ln-exp batch
    NG = JT // CH

    with tile.TileContext(nc) as tc, ExitStack() as ctx:
        xp = ctx.enter_context(tc.tile_pool(name="xp", bufs=1))
        sp = ctx.enter_context(tc.tile_pool(name="sp", bufs=1))
        small = ctx.enter_context(tc.tile_pool(name="small", bufs=1))
        psum = ctx.enter_context(tc.tile_pool(name="psum", bufs=1, space="PSUM"))

        # prime the ACT exp/ln table while DMAs stream
        pr = small.tile([128, 1], F32, tag="pr")
        nc.vector.memset(pr[:], 1.0)
        pr2 = small.tile([128, 1], F32, tag="pr2")
        nc.scalar.activation(pr2[:], pr[:], AF.Ln)

        # raw shards in CH-tile group DMAs (HWDGE has ~625ns fixed cost per
        # descriptor, so few big transfers beat 16 small ones); everything
        # rides the SP-hosted queue so the ACT sequencer stays free for the
        # ln/exp chain
        xg, yg = [], []
        for g in range(NG):
            t = xp.tile([128, CH * D], F16, tag=f"xg{g}")
            nc.sync.dma_start(t[:], xin[:, CH * D * g : CH * D * (g + 1)])
            xg.append(t)
            t = xp.tile([128, CH * D], F16, tag=f"yg{g}")
            nc.sync.dma_start(t[:], yin[:, CH * D * g : CH * D * (g + 1)])
            yg.append(t)

        def xt(j):
            return xg[j // CH][:, D * (j % CH) : D * (j % CH + 1)]

        def yt(j):
            return yg[j // CH][:, D * (j % CH) : D * (j % CH + 1)]

        # scaled tiles with a trailing ones column (gives row sums via the
        # same Gram matmuls); one strided memset per group covers the four
        # ones-columns without clogging the DVE sequencer
        xsg, ysg = [], []
        for g in range(NG):
            t = sp.tile([128, CH * W], F16, tag=f"xsg{g}")
            nc.vector.memset(
                t[:].rearrange("p (c w) -> p c w", c=CH)[:, :, D : D + 1], 1.0
            )
            xsg.append(t)
            t = sp.tile([128, CH * W], F16, tag=f"ysg{g}")
            nc.vector.memset(
                t[:].rearrange("p (c w) -> p c w", c=CH)[:, :, D : D + 1], 1.0
            )
            ysg.append(t)
        xs = [xsg[j // CH][:, W * (j % CH) : W * (j % CH + 1)] for j in range(JT)]
        ys = [ysg[j // CH][:, W * (j % CH) : W * (j % CH + 1)] for j in range(JT)]

        # PE p-state warmup during the DMA wait: ~2us of junk matmuls start
        # the clock ramp (0.65 -> 2.4 GHz needs ~3us of continuous busy)
        wsrc = small.tile([128, 512], F16, tag="wsrc")
        nc.vector.memset(wsrc[:], 0.0)
        wps = psum.tile([128, 512], F32, tag="wps")
        for w in range(5):
            nc.tensor.matmul(
                wps[:], lhsT=wsrc[:, 0:128], rhs=wsrc[:], start=True, stop=True,
                skip_group_check=True,
            )

        nsqx = small.tile([128, JT], F32, tag="nsqx")
        nsqy = small.tile([128, JT], F32, tag="nsqy")
        lnx = small.tile([128, JT], F32, tag="lnx")
        lny = small.tile([128, JT], F32, tag="lny")
        invx = small.tile([128, JT], F32, tag="invx")
        invy = small.tile([128, JT], F32, tag="invy")
        dots = small.tile([128, NG], F32, tag="dots")

        gx0 = psum.tile([128, W], F32, tag="gx0")
        gx1 = psum.tile([128, W], F32, tag="gx1")
        gy0 = psum.tile([128, W], F32, tag="gy0")
        gy1 = psum.tile([128, W], F32, tag="gy1")

        nscr = small.tile([128, D], F16, tag="nscr")

        def _scale(j, dst, srcfn, inv):
            # split the 32 row-scalings across DVE (16) / ACT (8) / Pool (8)
            if j % 4 == 0:
                nc.scalar.activation(
                    dst[:, 0:D], srcfn(j), AF.Copy, scale=inv[:, j : j + 1]
                )
            else:
                nc.vector.tensor_scalar(
                    dst[:, 0:D], srcfn(j), inv[:, j : j + 1], None, op0=MULT
                )

        def _norm(j, srcfn, nsq):
            # row sums of squares: x*x with add-accumulate (DVE ts-pow is
            # rejected by the ISA), split DVE (16) / Pool (8) / ACT (8)
            if j % 4 == 3:
                scr = small.tile([128, D], F16, tag=f"nsA{j % 2}")
                nc.scalar.activation(
                    scr[:], srcfn(j), AF.Square, accum_out=nsq[:, j : j + 1]
                )
            else:
                nc.vector.scalar_tensor_tensor(
                    nscr[:], srcfn(j), 1.0, srcfn(j), op0=MULT, op1=MULT,
                    accum_out=nsq[:, j : j + 1],
                )

        for g in range(NG):
            j0 = CH * g
            for j in range(j0, j0 + CH):
                _norm(j, xt, nsqx)
                _norm(j, yt, nsqy)
            cs = slice(j0, j0 + CH)
            nc.scalar.activation(lnx[:, cs], nsqx[:, cs], AF.Ln)
            nc.scalar.activation(invx[:, cs], lnx[:, cs], AF.Exp, scale=-0.5)
            nc.scalar.activation(lny[:, cs], nsqy[:, cs], AF.Ln)
            nc.scalar.activation(invy[:, cs], lny[:, cs], AF.Exp, scale=-0.5)
            for j in range(j0, j0 + CH):
                _scale(j, xs[j], xt, invx)
                _scale(j, ys[j], yt, invy)
                st = j == 0
                sp_ = j == JT - 1
                nc.tensor.matmul(
                    gx0[:], lhsT=xs[j][:, 0:128], rhs=xs[j], start=st, stop=sp_
                )
                nc.tensor.matmul(
                    gx1[:], lhsT=xs[j][:, 128:256], rhs=xs[j], start=st, stop=sp_
                )
                nc.tensor.matmul(
                    gy0[:], lhsT=ys[j][:, 0:128], rhs=ys[j], start=st, stop=sp_
                )
                nc.tensor.matmul(
                    gy1[:], lhsT=ys[j][:, 128:256], rhs=ys[j], start=st, stop=sp_
                )
            # exact diagonal partials xhat_i . yhat_i: one whole-group stt on
            # the Pool engine (the CH ones-columns add exactly CH per row,
            # which the host subtracts)
            dscr = small.tile([128, CH * W], F16, tag=f"dsc{g % 2}")
            nc.vector.scalar_tensor_tensor(
                dscr[:], xsg[g][:], 1.0, ysg[g][:],
                op0=MULT, op1=MULT, accum_out=dots[:, g : g + 1],
            )

        gx_sb = sp.tile([128, 2 * W], F32, tag="gxsb")
        nc.scalar.activation(gx_sb[:, 0:W], gx0[:], AF.Copy)
        nc.vector.tensor_copy(gx_sb[:, W : 2 * W], gx1[:])
        gy_sb = sp.tile([128, 2 * W], F32, tag="gysb")
        nc.scalar.activation(gy_sb[:, 0:W], gy0[:], AF.Copy)
        nc.vector.tensor_copy(gy_sb[:, W : 2 * W], gy1[:])

        nc.sync.dma_start(dto[:], dots[:])
        nc.sync.dma_start(
            gxo[:].rearrange("(h p) w -> p h w", p=128),
            gx_sb[:].rearrange("p (h w) -> p h w", h=2),
        )
        nc.scalar.dma_start(
            gyo[:].rearrange("(h p) w -> p h w", p=128),
            gy_sb[:].rearrange("p (h w) -> p h w", h=2),
        )

    nc.compile()
    return nc


def _get_moment_nc():
    global _CACHED_MOMENT_NC
    if _CACHED_MOMENT_NC is None:
        _CACHED_MOMENT_NC = _build_moment_nc()
    return _CACHED_MOMENT_NC


def _moment_in_maps(img, txt):
    img16 = np.asarray(img, dtype=np.float16)
    txt16 = np.asarray(txt, dtype=np.float16)

    def _pmajor(a):
        # [2048, 256] -> [128 partitions, 16 tiles * 256] tile-major per row
        return np.ascontiguousarray(
            a.reshape(JT, 128, D).transpose(1, 0, 2).reshape(128, JT * D)
        )

    in_maps = []
    for c in range(CORES):
        sl = slice(SH * c, SH * (c + 1))
        in_maps.append(
            {"xin": _pmajor(img16[sl]), "yin": _pmajor(txt16[sl])}
        )
    return in_maps


def _cheb_fit(s, b):
    """Degree-2 Chebyshev fit of softplus(s*c+b) over c in [-R, R].

    Returns power-basis coeffs (a0, a1, a2) and the measured max abs
    error eps on a dense grid (scaled by a small safety factor).
    """
    import numpy.polynomial.chebyshev as C

    R = _CHEB_R
    grid = np.linspace(-R, R, 20001)
    h = np.logaddexp(0.0, s * grid + b)
    cf = C.chebfit(grid / R, h, 2)
    eps = float(np.max(np.abs(C.chebval(grid / R, cf) - h))) * 1.05
    p = C.cheb2poly(cf)
    a0 = float(p[0])
    a1 = float(p[1]) / R
    a2 = float(p[2]) / R ** 2 if len(p) > 2 else 0.0
    return a0, a1, a2, eps


def _run_moment(img, txt, t_prime, bias, trace=False):
    s = float(np.exp(np.float64(np.asarray(t_prime, dtype=np.float32))))
    b = float(np.asarray(bias, dtype=np.float32))
    a0, a1, a2, eps = _cheb_fit(s, b)

    nc = _get_moment_nc()
    in_maps = _moment_in_maps(img, txt)
    res = run_bass_kernel_spmd(
        nc, in_maps, core_ids=list(range(CORES)), trace=trace
    )
    Gx = np.zeros((D, D), dtype=np.float64)
    Gy = np.zeros((D, D), dtype=np.float64)
    u = np.zeros(D, dtype=np.float64)
    v = np.zeros(D, dtype=np.float64)
    diag = 0.0
    for r in res.results:
        gxo = np.asarray(r["gxo"], dtype=np.float64)
        gyo = np.asarray(r["gyo"], dtype=np.float64)
        Gx += gxo[:, :D]
        Gy += gyo[:, :D]
        u += gxo[:, D]
        v += gyo[:, D]
        dto = np.asarray(r["dto"], dtype=np.float64)
        diag += float(dto.sum()) - 4.0 * dto.size  # remove ones-col dot
    S1 = float(u @ v)
    S2 = float(np.einsum("ij,ij->", Gx, Gy))
    total = a0 * N * N + a1 * S1 + a2 * S2 - (s * diag + N * b)
    loss = np.float32(total / N)
    # certified bound: fit error alone, over all N^2 pairs
    bound_rel = eps * N / max(abs(float(loss)), 1e-30)
    return loss, bound_rel, res


# ---------------------------------------------------------------------------
# exact path (fallback): exp -> pair-product tree -> ln, full N x SH logits
# ---------------------------------------------------------------------------
def _build_exact_nc():
    nc = bacc.Bacc(
        "TRN2",
        target_bir_lowering=False,
        debug=False,
        enable_asserts=False,
        num_devices=CORES,
    )
    txtT = nc.dram_tensor("txtT", [D, N], F16, kind="ExternalInput").ap()
    txtRF = nc.dram_tensor("txtRF", [N, D], F16, kind="ExternalInput").ap()
    txtRsh = nc.dram_tensor("txtRsh", [SH, D], F16, kind="ExternalInput").ap()
    imgT = nc.dram_tensor("imgT", [D, SH], F16, kind="ExternalInput").ap()
    imgR = nc.dram_tensor("imgR", [SH, D], F16, kind="ExternalInput").ap()
    tp = nc.dram_tensor("tp", [128, 1], F32, kind="ExternalInput").ap()
    bs = nc.dram_tensor("bs", [128, 1], F32, kind="ExternalInput").ap()
    out = nc.dram_tensor("out", [1, 1], F32, kind="ExternalOutput").ap()

    with tile.TileContext(nc) as tc, ExitStack() as ctx:
        big = ctx.enter_context(tc.tile_pool(name="big", bufs=1))
        rows = ctx.enter_context(tc.tile_pool(name="rows", bufs=32))
        scrp = ctx.enter_context(tc.tile_pool(name="scrp", bufs=4))
        ep = ctx.enter_context(tc.tile_pool(name="ep", bufs=3))
        uvp = ctx.enter_context(tc.tile_pool(name="uvp", bufs=3))
        lnp = ctx.enter_context(tc.tile_pool(name="lnp", bufs=2))
        small = ctx.enter_context(tc.tile_pool(name="small", bufs=1))
        psum = ctx.enter_context(tc.tile_pool(name="psum", bufs=2, space="PSUM"))

        # ---- resident loads --------------------------------------------
        tp_sb = small.tile([128, 1], F32, tag="tp")
        nc.scalar.dma_start(tp_sb[:], tp[:])
        bs_sb = small.tile([128, 1], F32, tag="bs")
        nc.scalar.dma_start(bs_sb[:], bs[:])
        imgT_sb = []
        for k in range(2):
            t = big.tile([128, SH], F16, tag=f"imgT{k}")
            nc.scalar.dma_start(t[:], imgT[128 * k : 128 * (k + 1), :])
            imgT_sb.append(t)
        imgR_sb = []
        for j in range(MT):
            r = big.tile([128, D], F16, tag=f"imgR{j}")
            nc.sync.dma_start(r[:], imgR[128 * j : 128 * (j + 1), :])
            imgR_sb.append(r)
        txtT_sb = []
        for k, eng in ((0, nc.sync), (1, nc.scalar)):
            t = big.tile([128, N], F16, tag=f"txtT{k}")
            eng.dma_start(t[:], txtT[128 * k : 128 * (k + 1), :])
            txtT_sb.append(t)

        e_ap = small.tile([128, 1], F32, tag="eap")
        nc.scalar.activation(e_ap[:], tp_sb[:], AF.Exp)

        ones_col = small.tile([128, 1], F32, tag="onesc")
        nc.vector.memset(ones_col[:], 1.0)
        ones_row16 = small.tile([1, 128], F16, tag="onesr16")
        nc.vector.memset(ones_row16[:], 1.0)
        ident = small.tile([128, 128], F32, tag="ident")
        from concourse.masks import make_identity

        make_identity(nc, ident[:])

        nsqx = small.tile([128, MT], F32, tag="nsqx")
        for j in range(MT):
            s1 = scrp.tile([128, D], F32, tag="dscr")
            nc.vector.scalar_tensor_tensor(
                s1[:], imgR_sb[j][:], 1.0, imgR_sb[j][:], op0=MULT, op1=MULT,
                accum_out=nsqx[:, j : j + 1],
            )
        lx = small.tile([128, MT], F32, tag="lx")
        nc.scalar.activation(lx[:], nsqx[:], AF.Ln)
        sxm = small.tile([128, MT], F32, tag="sxm")
        nc.scalar.activation(
            sxm[:], lx[:], AF.Exp, bias=tp_sb[:], scale=-0.5
        )
        ix = small.tile([128, MT], F32, tag="ix")
        nc.scalar.activation(ix[:], lx[:], AF.Exp, scale=-0.5)
        sxm_ps = psum.tile([16, 128], F32, tag="mm")
        nc.tensor.transpose(sxm_ps[:], sxm[:], ident[:])
        s16 = small.tile([16, 128], F16, tag="s16")
        nc.vector.tensor_copy(s16[:], sxm_ps[:])
        s1row = small.tile([1, SH], F16, tag="s1row")
        nc.gpsimd.dma_start(s1row[:], s16[:])
        sb_ps = psum.tile([128, SH], F32, tag="mm")
        for g in range(SH // 512):
            nc.tensor.matmul(
                sb_ps[:, 512 * g : 512 * (g + 1)],
                lhsT=ones_row16[:],
                rhs=s1row[:, 512 * g : 512 * (g + 1)],
                start=True,
                stop=True,
            )
        imgTs = []
        for k in range(2):
            t = big.tile([128, SH], F16, tag=f"imgTs{k}")
            nc.vector.tensor_tensor(t[:], imgT_sb[k][:], sb_ps[:], op=MULT)
            imgTs.append(t)

        NCHUNK = 16
        nsq = small.tile([128, NT], F32, tag="nsq")
        rinv_n = small.tile([128, NT], F32, tag="rinvn")
        invn = small.tile([128, NT], F32, tag="invn")
        acc = small.tile([128, NT // LN_BATCH], F32, tag="acc")
        half = SH // 2
        quart = SH // 4
        eighth = SH // 8
        lnt = None
        pending_ln = None

        def _norm_stt(u):
            rt = rows.tile([128, D], F16, tag="trow")
            nc.sync.dma_start(rt[:], txtRF[128 * u : 128 * (u + 1), :])
            scr = scrp.tile([128, D], F32, tag="ttrs")
            nc.vector.scalar_tensor_tensor(
                scr[:], rt[:], 1.0, rt[:], op0=MULT, op1=MULT,
                accum_out=nsq[:, u : u + 1],
            )

        def _finish_norm_chunk(c):
            cs = slice(NCHUNK * c, NCHUNK * (c + 1))
            nc.scalar.activation(rinv_n[:, cs], nsq[:, cs], AF.Ln)
            nc.scalar.activation(invn[:, cs], rinv_n[:, cs], AF.Exp, scale=-0.5)

        for u in range(NCHUNK):
            _norm_stt(u)
        _finish_norm_chunk(0)
        for t in range(NT):
            h = t % NCHUNK - 4
            c_next = t // NCHUNK + 1
            if 0 <= h < 8 and c_next < NT // NCHUNK:
                u0 = NCHUNK * c_next + 2 * h
                _norm_stt(u0)
                _norm_stt(u0 + 1)
            if t % NCHUNK == 12 and c_next < NT // NCHUNK:
                _finish_norm_chunk(c_next)
            ps = psum.tile([128, SH], F32, tag="mm")
            for k in range(2):
                for j in range(SH // 512):
                    sl = slice(512 * j, 512 * (j + 1))
                    nc.tensor.matmul(
                        ps[:, sl],
                        lhsT=txtT_sb[k][:, 128 * t : 128 * (t + 1)],
                        rhs=imgTs[k][:, sl],
                        start=(k == 0),
                        stop=(k == 1),
                    )
            et = ep.tile([128, SH], F16, tag="et")
            nc.scalar.activation(
                et[:], ps[:], AF.Exp, bias=bs_sb[:], scale=invn[:, t : t + 1]
            )
            if pending_ln is not None and t % LN_BATCH == 2:
                pending_ln()
                pending_ln = None
            if t % LN_BATCH == 0:
                lnt = lnp.tile([128, eighth * LN_BATCH], F16, tag="lnt")
            p = uvp.tile([128, SH], F16, tag="p")
            nc.vector.tensor_scalar(p[:], et[:], 1.0, 0.25, op0=ADD, op1=MULT)
            m1 = uvp.tile([128, half], F16, tag="m1")
            nc.vector.tensor_tensor(m1[:], p[:, 0:half], p[:, half:SH], op=MULT)
            m2 = uvp.tile([128, quart], F16, tag="m2")
            nc.vector.tensor_tensor(m2[:], m1[:, 0:quart], m1[:, quart:half], op=MULT)
            msl = lnt[:, eighth * (t % LN_BATCH) : eighth * (t % LN_BATCH + 1)]
            nc.vector.tensor_tensor(msl, m2[:, 0:eighth], m2[:, eighth:quart], op=MULT)
            if t % LN_BATCH == LN_BATCH - 1:
                b_idx = t // LN_BATCH
                cur_lnt = lnt

                def _emit_ln(cur_lnt=cur_lnt, b_idx=b_idx):
                    nc.scalar.activation(
                        cur_lnt[:],
                        cur_lnt[:],
                        AF.Ln,
                        accum_out=acc[:, b_idx : b_idx + 1],
                    )

                pending_ln = _emit_ln
        if pending_ln is not None:
            pending_ln()
            pending_ln = None

        nsqy = small.tile([128, MT], F32, tag="nsqy")
        dots = small.tile([128, MT], F32, tag="dots")
        for j in range(MT):
            tr = scrp.tile([128, D], F16, tag="srow_t")
            nc.sync.dma_start(tr[:], txtRsh[128 * j : 128 * (j + 1), :])
            s2 = scrp.tile([128, D], F32, tag="dscr")
            nc.vector.scalar_tensor_tensor(
                s2[:], tr[:], 1.0, tr[:], op0=MULT, op1=MULT,
                accum_out=nsqy[:, j : j + 1],
            )
            s3 = scrp.tile([128, D], F32, tag="dscr")
            nc.vector.scalar_tensor_tensor(
                s3[:], imgR_sb[j][:], 1.0, tr[:], op0=MULT, op1=MULT,
                accum_out=dots[:, j : j + 1],
            )
        ry = small.tile([128, MT], F32, tag="ry")
        nc.scalar.activation(ry[:], nsqy[:], AF.Ln)
        iy = small.tile([128, MT], F32, tag="iy")
        nc.scalar.activation(iy[:], ry[:], AF.Exp, scale=-0.5)
        sim = small.tile([128, MT], F32, tag="sim")
        nc.vector.tensor_tensor(sim[:], dots[:], ix[:], op=MULT)
        sim2 = small.tile([128, MT], F32, tag="sim2")
        nc.vector.tensor_tensor(sim2[:], sim[:], iy[:], op=MULT)
        dsum = small.tile([128, 1], F32, tag="dsum")
        nc.vector.reduce_sum(dsum[:], sim2[:], axis=mybir.AxisListType.X)

        A = small.tile([128, 1], F32, tag="A")
        nc.vector.reduce_sum(A[:], acc[:], axis=mybir.AxisListType.X)
        t1 = small.tile([128, 1], F32, tag="t1")
        nc.vector.tensor_tensor(t1[:], dsum[:], e_ap[:], op=MULT)
        t2 = small.tile([128, 1], F32, tag="t2")
        nc.vector.tensor_scalar(t2[:], bs_sb[:], float(MT), None, op0=MULT)
        t3 = small.tile([128, 1], F32, tag="t3")
        nc.vector.tensor_tensor(t3[:], t1[:], t2[:], op=ADD)
        corr = float((SH * N / 128 / 8) * 16.0 * np.log(2.0))
        A2 = small.tile([128, 1], F32, tag="A2")
        nc.vector.tensor_scalar(A2[:], A[:], corr, None, op0=ADD)
        C = small.tile([128, 1], F32, tag="C")
        nc.vector.tensor_tensor(C[:], A2[:], t3[:], op=SUB)
        fin_ps = psum.tile([1, 1], F32, tag="mm")
        nc.tensor.matmul(
            fin_ps[:], lhsT=ones_col[:], rhs=C[:], start=True, stop=True
        )
        fin = small.tile([1, 1], F32, tag="fin")
        nc.vector.tensor_copy(fin[:], fin_ps[:])
        nc.sync.dma_start(out[:], fin[:])

    nc.compile()
    return nc


def _get_exact_nc():
    global _CACHED_EXACT_NC
    if _CACHED_EXACT_NC is None:
        _CACHED_EXACT_NC = _build_exact_nc()
    return _CACHED_EXACT_NC


def _exact_in_maps(img, txt, t_prime, bias):
    img = np.asarray(img, dtype=np.float32)
    txt = np.asarray(txt, dtype=np.float32)
    tpv = float(np.asarray(t_prime, dtype=np.float32))
    bsv = float(np.asarray(bias, dtype=np.float32))

    txt16 = txt.astype(np.float16)
    txtT = np.ascontiguousarray(txt16.T)
    img16 = img.astype(np.float16)
    imgT = np.ascontiguousarray(img16.T)

    tp_arr = np.full((128, 1), tpv, dtype=np.float32)
    bs_arr = np.full((128, 1), bsv, dtype=np.float32)

    in_maps = []
    for c in range(CORES):
        sl = slice(SH * c, SH * (c + 1))
        in_maps.append(
            {
                "txtT": txtT,
                "txtRF": txt16,
                "txtRsh": np.ascontiguousarray(txt16[sl]),
                "imgT": np.ascontiguousarray(imgT[:, sl]),
                "imgR": np.ascontiguousarray(img16[sl]),
                "tp": tp_arr,
                "bs": bs_arr,
            }
        )
    return in_maps


def _run_exact(img, txt, t_prime, bias, trace=False):
    nc = _get_exact_nc()
    in_maps = _exact_in_maps(img, txt, t_prime, bias)
    res = run_bass_kernel_spmd(
        nc, in_maps, core_ids=list(range(CORES)), trace=trace
    )
    partials = [float(r["out"][0, 0]) for r in res.results]
    loss = np.float32(sum(partials) / N)
    return loss, res


# ---------------------------------------------------------------------------
# dispatcher
# ---------------------------------------------------------------------------
def _run(img, txt, t_prime, bias, trace=False):
    s = float(np.exp(np.float64(np.asarray(t_prime, dtype=np.float32))))
    b = float(np.asarray(bias, dtype=np.float32))
    _, _, _, eps = _cheb_fit(s, b)
    # pre-estimate of |loss| for path choice (softplus mean ~ a0, minus b)
    loss_scale = abs(N * float(np.logaddexp(0.0, b)) - b)
    if eps * N < _REL_GATE * max(loss_scale, 1e-30):
        loss, bound_rel, res = _run_moment(img, txt, t_prime, bias, trace=trace)
        if bound_rel < _REL_GATE:
            return loss, res
    return _run_exact(img, txt, t_prime, bias, trace=trace)


def kernel(img, txt, t_prime, bias):
    loss, _ = _run(img, txt, t_prime, bias, trace=False)
    return np.asarray(loss, dtype=np.float32)
